# revision 10
# baseline (speedup 1.0000x reference)
"""Gabor layer Trainium2 kernel — v3 "slot-stream" design.

Per gabor g, pixel (x,y): amp[g,c] * exp(E) * cos(S + phase[g,c]).
cos(S+p) = cos(p)cos(S) - sin(p)sin(S), so each channel sum over g is a
matmul over gauss*cos(S) / gauss*sin(S) planes (contraction = gabor slots).

A *slot* is a (tile, gabor) pair surviving a per-tile cull (tile = 16x32
pixels; keep if max_tile E clears the contribution threshold THR).  All
tiles share the same tile-local integer features / one-hot moving
operands, so slots from ANY tiles pack into one 128-partition plane:

  partitions 0:64   "sin" slots:  S  = A(row) + B(col)          (via matmul)
  partitions 64:128 "cos" slots:  S' = A(row) + wrap(B + pi/2)  (same slots)

One ACT pass over a plane then yields sin AND cos for 64 distinct slots
(ACT cost is free-dim only — partition parallelism is free), and ONE
K=128 reduce matmul per plane computes all packed tiles' channel sums
(alpha on cos rows, beta on sin rows), vs 4+ matmuls in the half-packed
v2 layout.

Planes are classified at pack time: a slot is *clean* if |A+B| and
|A+B'| stay within 0.98*pi over its tile (~79% of slots; the per-tile
phase span is small).  Clean planes feed Sin DIRECTLY from mS PSUM — no
DVE range-wrap at all.  Only wrappy planes pay the add_range_wrap.

E = quadratic in tile-local (dj,di) -> one f32r matmul (hi/lo split,
exact for pre-rounded inputs); Exp -> f16 gauss (duplicated top/bottom
via duplicated WE columns, free on PE).  ACT order [all Exp][all Sin] =
2 table loads.  Reduce outputs pack 3 jobs per PSUM bank at partition
bases {0,32,64} (matmul output base must be a 32-quadrant, 96 illegal),
30 rows each (10 tile-fragments x 3 channels, zero-padded); banks are
drained by a [96,N] copy + DMA; the host scatter-adds fragments.

Sharding: the global slot stream is split evenly across 8 cores (any
tile may fragment across planes/cores; host sums).  No collectives.
"""

import os
import sys

import numpy as np

for _p in ("/opt/trn_rl_repo",):
    if os.path.isdir(_p) and _p not in sys.path:
        sys.path.append(_p)

H = W = 512
G = 256
NCORES = 8
TR, TC = 16, 32       # tile rows x cols
N = TR * TC           # 512 pixels per tile
NTILES = (H // TR) * (W // TC)   # 512
KS = TR + TC          # one-hot rows: [row(16), col(32)] = 48
PI = float(np.pi)
THR = 6e-3            # per-tile cull threshold (error budget is 2e-2)
CLEAN_MARGIN = 0.98 * PI
JOB_M = 10            # tile fragments per reduce job (30 of 32 rows)
SIN_FLOOR_MS = 0.019  # ACT floor for the Sin phase (past the last Exp)

_PROGRAMS = {}


# ---------------------------------------------------------------------------
# Host-side parameter folding and per-tile analysis
# ---------------------------------------------------------------------------

def _wrap(x):
    return np.mod(x + np.pi, 2.0 * np.pi) - np.pi


def _to_f32r(a):
    b = np.ascontiguousarray(a, np.float32).view(np.uint32)
    r = (b + np.uint32(0x7FF) + ((b >> np.uint32(12)) & np.uint32(1))) \
        & np.uint32(0xFFFFF000)
    return r.view(np.float32)


def _to_bf16(a):
    import ml_dtypes
    return np.ascontiguousarray(np.asarray(a).astype(ml_dtypes.bfloat16))


def _fold_params(inputs):
    u = np.clip(np.asarray(inputs["u"], np.float64), -1, 1)
    v = np.clip(np.asarray(inputs["v"], np.float64), -1, 1)
    th = np.clip(np.asarray(inputs["theta"], np.float64), -2, 2) * (2 * np.pi)
    sig = np.clip(np.asarray(inputs["rel_sigma"], np.float64), 0.001, 1.0)
    rf = np.clip(np.asarray(inputs["rel_freq"], np.float64), -5, 5)
    gam = np.clip(np.asarray(inputs["gamma"], np.float64), 0.0001, 1.0)
    psi = np.clip(np.asarray(inputs["psi"], np.float64), -1, 1)
    amp = np.clip(np.asarray(inputs["amplitude"], np.float64), 0, 1)
    cr, sr = np.cos(th), np.sin(th)
    return dict(
        u=u, v=v, cr=cr, sr=sr,
        cx=-(cr * u + sr * v), cy=sr * u - cr * v,
        p=1.0 / (2.0 * sig * sig), q=1.0 / (2.0 * gam * gam),
        freq=2 * np.pi / np.exp(rf),
        alpha=amp * np.cos(psi * 2 * np.pi),
        beta=-amp * np.sin(psi * 2 * np.pi),
        amp=amp,
    )


def _tile_geometry(gx, gy):
    """Tile-major grids and per-tile affine centers/steps."""
    Xt = gx.reshape(H // TR, TR, W // TC, TC).transpose(0, 2, 1, 3).reshape(-1, N)
    Yt = gy.reshape(H // TR, TR, W // TC, TC).transpose(0, 2, 1, 3).reshape(-1, N)
    hx = Xt[:, 1] - Xt[:, 0]
    hy = Yt[:, TC] - Yt[:, 0]
    Xc = Xt[:, TR // 2 * TC + TC // 2]
    Yc = Yt[:, TR // 2 * TC + TC // 2]
    yrow = Yt.reshape(-1, TR, TC)[:, :, 0]
    xcol = Xt.reshape(-1, TR, TC)[:, 0, :]
    return Xc, Yc, hx, hy, yrow, xcol


def _we_coeffs(P, geo, tiles, gabors):
    """Quadratic E coefficients in tile-local integer coords for
    (tile, gabor) index arrays (broadcast to a common shape).
    E = w0*dj + w1*di + w2 + w3*dj^2 + w4*di^2 + w5*di*dj."""
    Xc, Yc, hx, hy, _, _ = geo
    cr = P["cr"][gabors]; sr = P["sr"][gabors]
    pk = P["p"][gabors]; qk = P["q"][gabors]
    XcT = Xc[tiles]; YcT = Yc[tiles]
    hxT = hx[tiles]; hyT = hy[tiles]
    cxt = XcT * cr + YcT * sr + P["cx"][gabors]
    cyt = -XcT * sr + YcT * cr + P["cy"][gabors]
    a1 = hxT * cr; a2 = hyT * sr
    b1 = -hxT * sr; b2 = hyT * cr
    w = np.empty((6,) + np.broadcast(cxt, cyt).shape)
    w[0] = -2.0 * (pk * cxt * a1 + qk * cyt * b1)
    w[1] = -2.0 * (pk * cxt * a2 + qk * cyt * b2)
    w[2] = -(pk * cxt * cxt + qk * cyt * cyt)
    w[3] = -(pk * a1 * a1 + qk * b1 * b1)
    w[4] = -(pk * a2 * a2 + qk * b2 * b2)
    w[5] = -2.0 * (pk * a1 * a2 + qk * b1 * b2)
    return w, cxt


def _box_max_E(w):
    """Max over the tile box of the concave quadratic E (continuous
    relaxation — conservative for culling).  w: [6, T, G]."""
    jlo, jhi = -(TC // 2), TC // 2 - 1
    ilo, ihi = -(TR // 2), TR // 2 - 1

    def ev(dj, di):
        return (w[0] * dj + w[1] * di + w[2] + w[3] * dj * dj
                + w[4] * di * di + w[5] * di * dj)

    best = np.full(w.shape[1:], -np.inf)
    # interior critical point: solve [2w3, w5; w5, 2w4] [dj,di] = -[w0,w1]
    det = 4.0 * w[3] * w[4] - w[5] * w[5]
    safe = np.abs(det) > 1e-30
    dj0 = np.where(safe, (-w[0] * 2.0 * w[4] + w[1] * w[5]) / np.where(safe, det, 1), 0.0)
    di0 = np.where(safe, (-w[1] * 2.0 * w[3] + w[0] * w[5]) / np.where(safe, det, 1), 0.0)
    inside = safe & (dj0 >= jlo) & (dj0 <= jhi) & (di0 >= ilo) & (di0 <= ihi)
    v = ev(dj0, di0)
    best = np.where(inside, np.maximum(best, v), best)
    # edges: dj fixed -> 1D concave in di (critical clamped); di fixed sym.
    for dj in (jlo, jhi):
        a = w[4]; b = w[1] + w[5] * dj
        di_c = np.where(np.abs(a) > 1e-30, -b / (2.0 * np.where(np.abs(a) > 1e-30, a, 1)), 0.0)
        di_c = np.clip(di_c, ilo, ihi)
        for di in (ilo, ihi):
            best = np.maximum(best, ev(dj, di))
        best = np.maximum(best, ev(dj, di_c))
    for di in (ilo, ihi):
        a = w[3]; b = w[0] + w[5] * di
        dj_c = np.where(np.abs(a) > 1e-30, -b / (2.0 * np.where(np.abs(a) > 1e-30, a, 1)), 0.0)
        dj_c = np.clip(dj_c, jlo, jhi)
        best = np.maximum(best, ev(dj_c, di))
    return best


def _slot_tables(P, geo, t_arr, g_arr):
    """Per-slot WE [6,k], A [k,16], B [k,32], B2 [k,32] (float64)."""
    Xc, Yc, hx, hy, yrow, xcol = geo
    cr = P["cr"][g_arr]; sr = P["sr"][g_arr]
    fk = P["freq"][g_arr]
    w, cxt = _we_coeffs(P, geo, t_arr, g_arr)
    A = _wrap(fk[:, None] * sr[:, None]
              * (yrow[t_arr] - Yc[t_arr][:, None]))
    Braw = (fk[:, None] * cr[:, None] * (xcol[t_arr] - Xc[t_arr][:, None])
            + (fk * cxt)[:, None])
    B = _wrap(Braw)
    B2 = _wrap(Braw + np.pi / 2)
    return w, A, B, B2


# ---------------------------------------------------------------------------
# Packing: slots -> planes -> jobs
# ---------------------------------------------------------------------------

def _pack_stream(stream, frag_cap):
    """stream: list of (tile, gabor) tile-major.  Planes of <=64 slots
    spanning <= frag_cap distinct tiles."""
    planes = []
    cur, cur_tiles = [], []
    for (t, g) in stream:
        new_tile = (not cur_tiles) or cur_tiles[-1] != t
        if len(cur) == 64 or (new_tile and len(cur_tiles) == frag_cap):
            planes.append(cur)
            cur, cur_tiles = [], []
            new_tile = True
        cur.append((t, g))
        if new_tile or cur_tiles[-1] != t:
            cur_tiles.append(t)
    if cur:
        planes.append(cur)
    return planes


def _prepare(inputs):
    """Full host-side prep.  Returns (in_maps, meta) where meta carries
    the program-shape params and the host reassembly map."""
    gx = np.asarray(inputs["grid_x"], np.float64)
    gy = np.asarray(inputs["grid_y"], np.float64)
    P = _fold_params(inputs)
    geo = _tile_geometry(gx, gy)

    # --- per (tile, gabor) cull + cleanliness ---
    tiles_b = np.arange(NTILES)[:, None]
    gab_b = np.arange(G)[None, :]
    w_all, _ = _we_coeffs(P, geo, tiles_b, gab_b)       # [6, T, G]
    Em = _box_max_E(w_all)                               # [T, G]
    ampmax = P["amp"].max(1)
    elim = np.log(np.maximum(THR / np.maximum(ampmax, 1e-30), 1e-300)) - 1.0
    keep = Em >= elim[None, :]                           # [T, G]
    t_idx, g_idx = np.nonzero(keep)
    nslots = len(t_idx)

    # clean test: max |A+B| and |A+B2| over the tile
    _, A, B, B2 = _slot_tables(P, geo, t_idx, g_idx)
    sm = np.maximum(A.max(1)[:, None] + np.stack([B.max(1), B2.max(1)], 1),
                    -(A.min(1)[:, None] + np.stack([B.min(1), B2.min(1)], 1)))
    clean = (sm.max(1) <= CLEAN_MARGIN)

    # --- shard the clean / wrappy streams across cores (tile-major) ---
    def shard(mask):
        order = np.lexsort((g_idx[mask], t_idx[mask]))
        sl = [(int(t_idx[mask][i]), int(g_idx[mask][i])) for i in order]
        chunks = []
        base = 0
        for c in range(NCORES):
            n = (len(sl) - base + (NCORES - 1 - c)) // (NCORES - c)
            chunks.append(sl[base:base + n])
            base += n
        return chunks

    clean_chunks = shard(clean)
    wrap_chunks = shard(~clean)

    core_planes = []
    for c in range(NCORES):
        wp = _pack_stream(wrap_chunks[c], 2 * JOB_M)
        cp = _pack_stream(clean_chunks[c], JOB_M)
        core_planes.append((wp, cp))
    PW = max(len(wp) for wp, _ in core_planes)
    PC = max(len(cp) for _, cp in core_planes)
    PW = 2 * ((PW + 1) // 2)      # pair-align the wrap path
    if (PW + PC) % 2:
        PC += 1
    pairs = (PW + PC) // 2
    pwp = PW // 2                 # pairs on the wrap path
    Ptot = 2 * pairs
    # one reduce job per plane: wrappy planes get a wide (M=60) job in an
    # exclusive PSUM bank at base 0; clean jobs pack 3 per bank at bases
    # {0,32,64} (matmul output base must be a 32-quadrant)
    rj = Ptot
    nb = PW + (Ptot - PW + 2) // 3

    in_maps = []
    host_map = []   # per core: list over jobs of list of tiles (frag order)
    for c in range(NCORES):
        wp, cp = core_planes[c]
        planes = wp + [[]] * (PW - len(wp)) + cp + [[]] * (Ptot - PW - len(cp))
        ws_h = np.zeros((pairs, KS, 2, 128), np.float32)
        we_h = np.zeros((pairs, 12, 2, 128), np.float32)
        ab_h = np.zeros((128, rj, 6 * JOB_M))
        jobs_tiles = []
        bank, reg = 0, 0
        for pl in range(Ptot):
            slots = planes[pl]
            pr, hh = divmod(pl, 2)
            if slots:
                t_a = np.array([s[0] for s in slots])
                g_a = np.array([s[1] for s in slots])
                k = len(slots)
                w6, Asl, Bsl, B2sl = _slot_tables(P, geo, t_a, g_a)
                WEh = _to_f32r(w6)                     # [6, k]
                WEl = _to_f32r(w6 - WEh)
                for off in (0, 64):
                    we_h[pr, 0:6, hh, off:off + k] = WEh
                    we_h[pr, 6:12, hh, off:off + k] = WEl
                for off, Bv in ((0, Bsl), (64, B2sl)):
                    WS = np.concatenate([Asl.T, Bv.T], 0)     # [48, k]
                    ws_h[pr, :, hh, off:off + k] = _to_f32r(WS)
            frags = []
            for j, (t, g) in enumerate(slots):
                if not frags or frags[-1][0] != t:
                    frags.append((t, []))
                frags[-1][1].append((j, g))
            for f, (t, members) in enumerate(frags):
                for (j, g) in members:
                    ab_h[j, pl, 3 * f:3 * f + 3] = P["beta"][g]
                    ab_h[64 + j, pl, 3 * f:3 * f + 3] = P["alpha"][g]
            if pl < PW:
                if reg:
                    bank += 1
                    reg = 0
                jobs_tiles.append((bank, 0, [t for t, _ in frags]))
                bank += 1
            else:
                jobs_tiles.append((bank, 32 * reg, [t for t, _ in frags]))
                reg += 1
                if reg == 3:
                    bank, reg = bank + 1, 0
        assert len(jobs_tiles) == rj
        ii, jj = np.divmod(np.arange(N), TC)
        di = (ii - TR // 2).astype(np.float64)
        dj = (jj - TC // 2).astype(np.float64)
        feat6 = np.stack([dj, di, np.ones_like(dj), dj * dj, di * di, dj * di], 0)
        feat12 = np.concatenate([feat6, feat6], 0).astype(np.float32)
        onehot = np.zeros((KS, N), np.float32)
        onehot[ii, np.arange(N)] = 1.0
        onehot[TR + jj, np.arange(N)] = 1.0
        in_maps.append({
            "feat": feat12,
            "onehot": onehot,
            "we": np.ascontiguousarray(we_h.transpose(1, 0, 2, 3)
                                       .reshape(12, pairs, 2 * 128)),
            "ws": np.ascontiguousarray(ws_h.transpose(1, 0, 2, 3)
                                       .reshape(KS, pairs, 2 * 128)),
            "ab": ab_h.astype(np.float16),
        })
        host_map.append(jobs_tiles)

    meta = dict(pairs=pairs, pwp=pwp, rj=rj, nb=nb, host_map=host_map)
    return in_maps, meta


# ---------------------------------------------------------------------------
# Device program
# ---------------------------------------------------------------------------

def _build_program(pairs, pwp, rj):
    from concourse import bacc, mybir, tile

    f32 = mybir.dt.float32
    f32r = mybir.dt.float32r
    bf16 = mybir.dt.bfloat16
    f16 = mybir.dt.float16
    Act = mybir.ActivationFunctionType
    PW = 2 * pwp         # planes 0..PW-1 take the wrap path
    nb = PW + (rj - PW + 2) // 3

    nc = bacc.Bacc("TRN2", target_bir_lowering=False, debug=False,
                   num_devices=NCORES)

    featd = nc.dram_tensor("feat", [12, N], f32r, kind="ExternalInput")
    ohd = nc.dram_tensor("onehot", [KS, N], f32r, kind="ExternalInput")
    wed = nc.dram_tensor("we", [12, pairs, 2 * 128], f32r, kind="ExternalInput")
    wsd = nc.dram_tensor("ws", [KS, pairs, 2 * 128], f32r, kind="ExternalInput")
    abd = nc.dram_tensor("ab", [128, rj, 6 * JOB_M], f16, kind="ExternalInput")
    outd = nc.dram_tensor("out", [nb, 96, N], f16, kind="ExternalOutput")

    with tile.TileContext(nc) as tc:
        with (
            tc.tile_pool(name="io", bufs=1) as iop,
            tc.tile_pool(name="gauss", bufs=pairs + 1) as gp,
            tc.tile_pool(name="w1s", bufs=pwp + 1) as w1p,
            tc.tile_pool(name="trig", bufs=3) as trigp,
            tc.tile_pool(name="prod", bufs=3) as pp,
            tc.tile_pool(name="mm", bufs=3, space="PSUM") as mmp,
            tc.tile_pool(name="po", bufs=2, space="PSUM") as pop,
        ):
            # first compute needs ft + we[0] (Exp path) and ws[0] + oh
            # (wrap path) — ship those before everything else
            ft_sb = iop.tile([12, N], f32r, tag="ft")
            nc.sync.dma_start(out=ft_sb[:], in_=featd[:])
            oh_sb = iop.tile([KS, N], f32r, tag="oh")
            nc.gpsimd.dma_start(out=oh_sb[:], in_=ohd[:])
            wes, wss = [], []
            chunks = [(0, 1), (1, 2)]
            base = 2
            while base < pairs:
                chunks.append((base, min(base + 4, pairs)))
                base += 4
            nwec = len(chunks)
            for (lo, hi) in chunks:
                npr = hi - lo
                we_c = iop.tile([12, npr, 2, 128], f32r, tag=f"we{npr}",
                                bufs=nwec, name="we_c")
                nc.sync.dma_start(out=we_c[:], in_=wed[:, lo:hi])
                ws_c = iop.tile([KS, npr, 2, 128], f32r, tag=f"ws{npr}",
                                bufs=nwec, name="ws_c")
                nc.gpsimd.dma_start(out=ws_c[:], in_=wsd[:, lo:hi])
                for pr in range(lo, hi):
                    wes.append(we_c[:, pr - lo])
                    wss.append(ws_c[:, pr - lo])
                if lo == 1:
                    ab_sb = iop.tile([128, rj, 6 * JOB_M], f16, tag="ab")
                    nc.sync.dma_start(out=ab_sb[:], in_=abd[:])

            # Phase A: all Exps (one table set); wrappy mS + DVE wraps overlap
            gts, w1s = [], []
            for i in range(pairs):
                if i < pwp:
                    mS = mmp.tile([128, 2, N], f32, tag="mm", name="mS")
                    nc.tensor.matmul(mS[:, 0, :], wss[i][:, 0, :], oh_sb[:],
                                     start=True, stop=True)
                    nc.tensor.matmul(mS[:, 1, :], wss[i][:, 1, :], oh_sb[:],
                                     start=True, stop=True)
                    w1 = w1p.tile([128, 2, N], f16, tag="w1", name="w1")
                    nc.vector.add_range_wrap(w1[:], mS[:], 0.0, PI, 2.0 * PI)
                    w1s.append(w1)
                mE = mmp.tile([128, 2, N], f32, tag="mm", name="mE")
                nc.tensor.matmul(mE[:, 0, :], wes[i][:, 0, :], ft_sb[:],
                                 start=True, stop=True)
                nc.tensor.matmul(mE[:, 1, :], wes[i][:, 1, :], ft_sb[:],
                                 start=True, stop=True)
                gq = gp.tile([128, 2, N], f16, tag="g", name="gauss")
                nc.scalar.activation(gq[:], mE[:], Act.Exp)
                gts.append(gq)

            # Phase B: all Sins (second table set), muls, reduces, drains.
            # Clean mS pairs are software-pipelined 3 deep so PE keeps the
            # Sin stream fed across the table-load transition.
            def emit_msc(i):
                mS = mmp.tile([128, 2, N], f32, tag="mm", name="mSc")
                nc.tensor.matmul(mS[:, 0, :], wss[i][:, 0, :], oh_sb[:],
                                 start=True, stop=True)
                nc.tensor.matmul(mS[:, 1, :], wss[i][:, 1, :], oh_sb[:],
                                 start=True, stop=True)
                return mS

            msc = {}
            for i in range(pwp, min(pwp + 3, pairs)):
                msc[i] = emit_msc(i)

            bank, reg = 0, 0
            po = None
            pending_drain = None
            for i in range(pairs):
                with tc.tile_wait_until(SIN_FLOOR_MS):
                    trig = trigp.tile([128, 2, N], f16, tag="tr", name="trig")
                    if i < pwp:
                        nc.scalar.activation(trig[:], w1s[i][:], Act.Sin)
                    else:
                        nc.scalar.activation(trig[:], msc.pop(i)[:], Act.Sin)
                nxt = max(i + 3, pwp + 3)
                if nxt < pairs and nxt not in msc:
                    msc[nxt] = emit_msc(nxt)
                pq = pp.tile([128, 2, N], f16, tag="pq", name="pq")
                nc.vector.tensor_mul(pq[:], gts[i][:], trig[:])
                for hh in range(2):
                    pl = 2 * i + hh

                    def drain(b):
                        ob = pp.tile([96, N], f16, tag="ob", name="ob")
                        if b >= nb - 2:
                            nc.scalar.copy(ob[:], po[0:96, :])
                        else:
                            nc.vector.tensor_copy(ob[:], po[0:96, :])
                        eng = nc.sync if b % 2 == 0 else nc.gpsimd
                        eng.dma_start(out=outd[b], in_=ob[:])

                    if pl < PW:      # wide wrappy job, exclusive bank
                        if reg:      # close a partial clean bank (unused)
                            drain(bank)
                            bank, reg = bank + 1, 0
                        po = pop.tile([128, N], f32, tag="po", name="po")
                        nc.tensor.matmul(
                            po[0:6 * JOB_M, :], ab_sb[:, pl, :],
                            pq[:, hh, :],
                            start=True, stop=True, skip_group_check=True,
                        )
                        drain(bank)
                        bank += 1
                    else:
                        if reg == 0:
                            po = pop.tile([128, N], f32, tag="po", name="po")
                        nc.tensor.matmul(
                            po[32 * reg:32 * reg + 3 * JOB_M, :],
                            ab_sb[:, pl, 0:3 * JOB_M], pq[:, hh, :],
                            start=True, stop=True, skip_group_check=True,
                        )
                        reg += 1
                        if reg == 3 or pl == rj - 1:
                            drain(bank)
                            bank, reg = bank + 1, 0

    nc.compile()
    return nc


# ---------------------------------------------------------------------------
# Entry point
# ---------------------------------------------------------------------------

def kernel(**inputs):
    from concourse.bass_utils import run_bass_kernel_spmd

    in_maps, meta = _prepare(inputs)
    key = (meta["pairs"], meta["pwp"], meta["rj"])
    if key not in _PROGRAMS:
        _PROGRAMS[key] = _build_program(*key)
    nc = _PROGRAMS[key]
    res = run_bass_kernel_spmd(nc, in_maps, list(range(NCORES)))

    out = np.zeros((3, H, W), np.float64)
    ntc = W // TC
    for c in range(NCORES):
        r = np.asarray(res.results[c]["out"], np.float64)   # [nb, 96, N]
        for (b, row0, tiles) in meta["host_map"][c]:
            for f, t in enumerate(tiles):
                tr_, tc_ = divmod(t, ntc)
                rows = r[b, row0 + 3 * f:row0 + 3 * f + 3]   # [3, N]
                out[:, tr_ * TR:(tr_ + 1) * TR, tc_ * TC:(tc_ + 1) * TC] += \
                    rows.reshape(3, TR, TC)
    return np.clip(out, -1.0, 1.0).astype(np.float32)
